# revision 5
# baseline (speedup 1.0000x reference)
"""Trainium2 Bass kernel for a 2-layer LSTM encoder returning final (h, c).

Problem: enc_inp [B=128, T=1024, F=64]; two stacked LSTM layers with H=128;
layer 2's initial state is layer 1's final state, so the 2048 recurrence
steps are strictly sequential and the kernel is bound by the per-step
cross-engine dependency chain  PE(U@h) -> ACT(sigmoid) -> DVE(cell) ->
ACT(tanh) -> DVE(h) -> PE.

Chain-latency optimizations vs the padded predecessor (HW-validated ~11%
faster in an interleaved A/B; sim + HW agree on the structure):
 - No filler/pad ops (Tile's scheduler displaced them from their intended
   per-step slots anyway, so they only added engine busy-work).
 - Gate column order (f, g2, i, o): only the f,g,i matmuls + one 48-wide
   sigmoid sit on the critical path. The o-gate matmul accumulates into its
   OWN psum bank (separate tile - avoids Tile's same-bank write-after-read
   serialization against the 3-gate sigmoid) and sigma(o) runs in the
   shadow of the DVE cell update.
 - tanh(g) via 2*sigmoid(2x)-1 with g-gate weights pre-scaled by 2 (host),
   so the gate activation is a single Sigmoid op; cell update is 3 fused
   DVE ops (stt/tt/stt); tanh(c) is the only other ACT op; h in bf16.
 - Static parity-pair (t%2) tiles for all per-step tensors instead of
   rotating pools: WAR/WAW deps resolve to ticks already covered by each
   engine's observed vector clock, so chain instructions carry their real
   cross-engine wait attached (fewer sequencer-blocking EventSemaphores).
Rejected on HW measurement: per-step psum tiles (bank cycling + extra
LDWEIGHTS cost ~+900ns/step), GPSIMD offload of fc/cell ops (stt fails to
compile; fc-on-pool measured +48ns/step in an interleaved A/B - GPSIMD
dispatch + the DVE-shared SBUF port cost more than the overlap saves),
SEQ-pacing pads, bank-interleaved step layout (sim-neutral).
"""

import numpy as np
import ml_dtypes

import concourse.bacc as bacc
import concourse.tile as tile
import concourse.mybir as mybir
from concourse.bass_utils import run_bass_kernel_spmd

N_CORES = 8
B, T_FULL, F, H = 128, 1024, 64, 128
BS = B // N_CORES
G4 = 4 * H

BF16 = ml_dtypes.bfloat16

# keras gate order (i, f, g, o) -> ours (f, g, i, o); g block scaled by 2
_PERM = np.concatenate(
    [np.arange(H, 2 * H), np.arange(2 * H, 3 * H), np.arange(0, H),
     np.arange(3 * H, 4 * H)]
)

_ALU = mybir.AluOpType
_ACT = mybir.ActivationFunctionType

DEFAULT_CFG = {
    "fc": "dve",       # "dve" | "pool"  (engine for fc = sigma(f)*c)
    "ho": "dve",       # "dve" | "pool"  (engine for h = tanh(c)*sigma(o))
    "cell": "dve",     # "dve" | "pool"  (engine for ig2/c/ho)
    "split_o": True,    # o-gate matmul + sigmoid off the critical path
    "static": True,     # parity-pair static tiles instead of pool rotation
    "touch": None,      # None | "dve": overwrite dead ACT-written tiles so
                        # the next ACT write's WAW dep is a covered cross tick
                        # (HW A/B inconclusive; default off)
    "bufs": 8,
    "chunk": 16,
    "qw": 64,
    "perstep": False,   # per-step psum z tiles (bank rotation) + lookahead xz
    "look": 4,          # xz lookahead steps in perstep mode
    "pz_bufs": 6,       # per-step z tile pool depth (each = 1 psum bank)
    "ilv": False,       # interleave steps across the 2 banks of the chunked
                        # pz tile (even steps bank0, odd bank1) so consecutive
                        # sigmas read different banks and the PE wait attaches
}


def _build(T, has_b1, reps=1, cfg=None):
    cfg = dict(DEFAULT_CFG, **(cfg or {}))
    CHUNK = cfg["chunk"]
    QW = cfg["qw"]
    bufs = cfg["bufs"]
    split_o = cfg["split_o"]
    fc_eng = cfg["fc"]
    cell_eng = cfg["cell"]
    static = cfg["static"]
    touch = cfg["touch"]
    perstep = cfg["perstep"]
    LOOK = cfg["look"]
    pz_bufs = cfg["pz_bufs"]
    ilv = cfg["ilv"]
    if perstep:
        assert split_o and pz_bufs >= LOOK + 2
    if ilv:
        assert split_o and CHUNK == 16 and BS == 16

    bf = mybir.dt.bfloat16
    f32 = mybir.dt.float32

    nc = bacc.Bacc("TRN2", target_bir_lowering=False, debug=False,
                   enable_asserts=True, num_devices=N_CORES)

    xT = nc.dram_tensor("xT", [F + 1, T * BS], bf, kind="ExternalInput").ap()
    w0 = nc.dram_tensor("w0", [F + 1, G4], bf, kind="ExternalInput").ap()
    u0 = nc.dram_tensor("u0", [H, G4], bf, kind="ExternalInput").ap()
    w1 = nc.dram_tensor("w1", [H, G4], bf, kind="ExternalInput").ap()
    u1 = nc.dram_tensor("u1", [H, G4], bf, kind="ExternalInput").ap()
    if has_b1:
        b1 = nc.dram_tensor("b1", [1, G4], bf, kind="ExternalInput").ap()
    hc = nc.dram_tensor("hc", [H, 2 * BS], f32, kind="ExternalOutput").ap()

    assert T % CHUNK == 0
    PZW = 4 * CHUNK * BS
    NPIECE = CHUNK
    PPG = CHUNK * BS // QW
    assert PPG * 4 == NPIECE

    with tile.TileContext(nc) as tc:
        with (
            tc.tile_pool(name="big", bufs=1) as big,
            tc.tile_pool(name="wts", bufs=1) as wts,
            tc.tile_pool(name="state", bufs=1) as state,
            tc.tile_pool(name="gates", bufs=bufs) as gates,
            tc.tile_pool(name="sop", bufs=bufs) as sop,
            tc.tile_pool(name="tmps", bufs=bufs) as tmps,
            tc.tile_pool(name="hsmall", bufs=bufs) as hsmall,
            tc.tile_pool(name="pz", bufs=2, space="PSUM") as pzpool,
            tc.tile_pool(name="pzo", bufs=2, space="PSUM") as pzopool,
            tc.tile_pool(name="pzs", bufs=pz_bufs, space="PSUM") as pzs,
        ):
            xTs = big.tile([F + 1, T * BS], bf, tag="xT")
            nc.sync.dma_start(out=xTs, in_=xT)
            hs0 = big.tile([H, T * BS], bf, tag="hs0")

            w0s = wts.tile([F + 1, G4], bf, tag="w0")
            u0s = wts.tile([H, G4], bf, tag="u0")
            w1s = wts.tile([H, G4], bf, tag="w1")
            u1s = wts.tile([H, G4], bf, tag="u1")
            nc.sync.dma_start(out=w0s, in_=w0)
            nc.sync.dma_start(out=u0s, in_=u0)
            nc.sync.dma_start(out=w1s, in_=w1)
            nc.sync.dma_start(out=u1s, in_=u1)
            b1s = None
            ones = None
            if has_b1:
                b1s = wts.tile([1, G4], bf, tag="b1")
                nc.sync.dma_start(out=b1s, in_=b1)
                ones = state.tile([1, BS], bf, tag="ones")
                nc.vector.memset(ones, 1.0)

            c = state.tile([H, BS], f32, tag="c")
            h0 = state.tile([H, BS], bf, tag="h0")
            hc_stage = state.tile([H, 2 * BS], f32, tag="hc_stage")

            fc_op = nc.gpsimd if fc_eng == "pool" else nc.vector
            cell_op = nc.gpsimd if cell_eng == "pool" else nc.vector
            ho_op = nc.gpsimd if cfg["ho"] == "pool" else nc.vector

            NG = 3 if split_o else 4   # gates in the main psum tile

            if static:
                # parity-pair static tiles: step t uses index t % 2
                S_ab = [state.tile([H, NG * BS], f32, tag=f"S{p}", name=f"S{p}")
                        for p in range(2)]
                So_ab = ([state.tile([H, BS], f32, tag=f"So{p}", name=f"So{p}")
                          for p in range(2)] if split_o else None)
                ig2_ab = [state.tile([H, BS], f32, tag=f"ig2{p}", name=f"ig2{p}")
                          for p in range(2)]
                fc_ab = [state.tile([H, BS], f32, tag=f"fc{p}", name=f"fc{p}")
                         for p in range(2)]
                th_ab = [state.tile([H, BS], f32, tag=f"th{p}", name=f"th{p}")
                         for p in range(2)]
                h1_ab = [state.tile([H, BS], bf, tag=f"h1{p}", name=f"h1{p}")
                         for p in range(2)]
            PZW_M = NG * CHUNK * BS

            def emit_gemm_piece(pzs, w_s, x_s, c0, piece):
                """pzs = (pz_main, pz_o or None)"""
                j, q = piece // PPG, piece % PPG
                cols = slice(c0 * BS + q * QW, c0 * BS + (q + 1) * QW)
                if split_o and j == 3:
                    dst = pzs[1][:, q * QW:(q + 1) * QW]
                    start = q == 0
                else:
                    pz3 = pzs[0].rearrange("p (g n) -> p g n", g=NG)
                    dst = pz3[:, j, q * QW:(q + 1) * QW]
                    start = q == 0 and j % 2 == 0
                nc.tensor.matmul(
                    dst, w_s[:, j * H:(j + 1) * H], x_s[:, cols],
                    start=start, stop=False, skip_group_check=True,
                )

            def alloc_pz():
                pz = pzpool.tile([H, PZW_M], f32, tag="pz")
                if split_o:
                    pzo = pzopool.tile([H, CHUNK * BS], f32, tag="pzo")
                else:
                    pzo = None
                return (pz, pzo)

            def emit_layer_perstep(layer, x_s, w_s, u_s, b_s, h_prev,
                                   last_layer):
                """Per-step z tiles in rotating psum banks; o-gate chunked."""
                zt = {}

                def emit_xz_step(tp):
                    z = pzs.tile([H, 3 * BS], f32, tag="z", name="z")
                    zt[tp] = z
                    for j in range(3):
                        nc.tensor.matmul(
                            z[:, j * BS:(j + 1) * BS],
                            w_s[:, j * H:(j + 1) * H],
                            x_s[:, tp * BS:(tp + 1) * BS],
                            start=(j == 0), stop=False, skip_group_check=True,
                        )

                def emit_xzo_chunk(pzo, c0):
                    for q in range(PPG):
                        cols = slice(c0 * BS + q * QW, c0 * BS + (q + 1) * QW)
                        nc.tensor.matmul(
                            pzo[:, q * QW:(q + 1) * QW],
                            w_s[:, 3 * H:4 * H], x_s[:, cols],
                            start=(q == 0), stop=False, skip_group_check=True,
                        )

                for tp in range(min(LOOK, T)):
                    emit_xz_step(tp)
                pzo_cur = None
                for t in range(T):
                    k = t % CHUNK
                    if k == 0:
                        pzo_cur = pzopool.tile([H, CHUNK * BS], f32,
                                               tag="pzo", name="pzo")
                        emit_xzo_chunk(pzo_cur, t)
                    sl = slice(k * BS, (k + 1) * BS)
                    z_cur = zt.pop(t)
                    for j in range(3):
                        nc.tensor.matmul(
                            z_cur[:, j * BS:(j + 1) * BS],
                            u_s[:, j * H:(j + 1) * H],
                            h_prev,
                            start=False, stop=(j == 2 and b_s is None),
                            skip_group_check=True,
                        )
                        if b_s is not None:
                            nc.tensor.matmul(
                                z_cur[:, j * BS:(j + 1) * BS],
                                b_s[:, j * H:(j + 1) * H], ones,
                                start=False, stop=(j == 2),
                                skip_group_check=True,
                            )
                    if static:
                        S = S_ab[t % 2]
                    else:
                        S = gates.tile([H, 3 * BS], f32, tag="S", name="S")
                    nc.scalar.activation(S, z_cur, _ACT.Sigmoid)
                    sf = S[:, 0:BS]
                    sg = S[:, BS:2 * BS]
                    si = S[:, 2 * BS:3 * BS]
                    # o-gate matmul + sigma(o), off the chain
                    zo = pzo_cur[:, sl]
                    nc.tensor.matmul(
                        zo, u_s[:, 3 * H:4 * H], h_prev,
                        start=False, stop=b_s is None, skip_group_check=True,
                    )
                    if b_s is not None:
                        nc.tensor.matmul(
                            zo, b_s[:, 3 * H:4 * H], ones,
                            start=False, stop=True, skip_group_check=True,
                        )
                    if static:
                        So = So_ab[t % 2]
                    else:
                        So = sop.tile([H, BS], f32, tag="So", name="So")
                    nc.scalar.activation(So, zo, _ACT.Sigmoid)
                    # xz for step t+LOOK
                    if t + LOOK < T:
                        emit_xz_step(t + LOOK)
                    # cell update
                    if static:
                        ig2 = ig2_ab[t % 2]
                        fcv = fc_ab[t % 2]
                        th = th_ab[t % 2]
                    else:
                        ig2 = tmps.tile([H, BS], f32, tag="ig2", name="ig2")
                        fcv = tmps.tile([H, BS], f32, tag="fc", name="fc")
                        th = tmps.tile([H, BS], f32, tag="th", name="th")
                    cell_op.scalar_tensor_tensor(
                        ig2, sg, 0.5, si, _ALU.subtract, _ALU.mult)
                    fc_op.tensor_mul(fcv, sf, c)
                    cell_op.scalar_tensor_tensor(
                        c, ig2, 2.0, fcv, _ALU.mult, _ALU.add)
                    nc.scalar.activation(th, c, _ACT.Tanh)
                    last_step = last_layer and t == T - 1
                    if last_step:
                        ho_op.tensor_mul(hc_stage[:, 0:BS], th, So)
                    else:
                        if layer == 0:
                            h_prev = hs0[:, t * BS:(t + 1) * BS]
                        elif static:
                            h_prev = h1_ab[t % 2]
                        else:
                            h_prev = hsmall.tile([H, BS], bf, tag="h1",
                                                 name="h1")
                        ho_op.tensor_mul(h_prev, th, So)
                    if touch and static and not last_step:
                        t_op = nc.gpsimd if touch == "pool" else nc.vector
                        t_op.memset(S, 0.0)
                        t_op.memset(th, 0.0)
                        t_op.memset(So, 0.0)
                return h_prev

            def emit_layer_ilv(layer, x_s, w_s, u_s, b_s, h_prev, last_layer):
                """Chunked pz with step-parity bank interleave.

                pz tile [H, 1024] = 2 psum banks; step k of the chunk lives at
                [par=k%2, slot=k//2, 0:48] so consecutive sigmas read
                different banks (no same-bank R-R serialization). xz for the
                next chunk is woven one piece per step (6 main + PPG o-gate
                pieces per chunk); each piece covers one (gate, parity).
                """
                NSLOT = CHUNK // 2

                def pz4(pz):
                    return pz.rearrange("p (par slot gb) -> p par slot gb",
                                        par=2, slot=NSLOT)

                def emit_piece(pzs_, c0, piece):
                    if piece < 6:
                        j, par = piece // 2, piece % 2
                        xv = x_s[:, c0 * BS:(c0 + CHUNK) * BS].rearrange(
                            "p (slot par b) -> p par slot b", par=2, b=BS)
                        nc.tensor.matmul(
                            pz4(pzs_[0])[:, par, :, j * BS:(j + 1) * BS],
                            w_s[:, j * H:(j + 1) * H],
                            xv[:, par],
                            start=(j == 0), stop=False, skip_group_check=True,
                        )
                    else:
                        q = piece - 6
                        cols = slice(c0 * BS + q * QW, c0 * BS + (q + 1) * QW)
                        nc.tensor.matmul(
                            pzs_[1][:, q * QW:(q + 1) * QW],
                            w_s[:, 3 * H:4 * H], x_s[:, cols],
                            start=(q == 0), stop=False, skip_group_check=True,
                        )

                NP = 6 + PPG

                def alloc_pz_ilv():
                    pz = pzpool.tile([H, 2 * NSLOT * 64], f32, tag="pz",
                                     name="pz")
                    pzo = pzopool.tile([H, CHUNK * BS], f32, tag="pzo",
                                       name="pzo")
                    return (pz, pzo)

                pz_cur = alloc_pz_ilv()
                for p in range(NP):
                    emit_piece(pz_cur, 0, p)
                pz_next = None
                for t in range(T):
                    k = t % CHUNK
                    if k == 0 and t > 0:
                        pz_cur = pz_next
                    zb = pz4(pz_cur[0])[:, k % 2, k // 2]
                    for j in range(3):
                        nc.tensor.matmul(
                            zb[:, j * BS:(j + 1) * BS],
                            u_s[:, j * H:(j + 1) * H],
                            h_prev,
                            start=False, stop=(j == 2 and b_s is None),
                            skip_group_check=True,
                        )
                        if b_s is not None:
                            nc.tensor.matmul(
                                zb[:, j * BS:(j + 1) * BS],
                                b_s[:, j * H:(j + 1) * H], ones,
                                start=False, stop=(j == 2),
                                skip_group_check=True,
                            )
                    if static:
                        S = S_ab[t % 2]
                    else:
                        S = gates.tile([H, 3 * BS], f32, tag="S", name="S")
                    nc.scalar.activation(S, zb[:, 0:3 * BS], _ACT.Sigmoid)
                    sf = S[:, 0:BS]
                    sg = S[:, BS:2 * BS]
                    si = S[:, 2 * BS:3 * BS]
                    zo = pz_cur[1][:, k * BS:(k + 1) * BS]
                    nc.tensor.matmul(
                        zo, u_s[:, 3 * H:4 * H], h_prev,
                        start=False, stop=b_s is None, skip_group_check=True,
                    )
                    if b_s is not None:
                        nc.tensor.matmul(
                            zo, b_s[:, 3 * H:4 * H], ones,
                            start=False, stop=True, skip_group_check=True,
                        )
                    if static:
                        So = So_ab[t % 2]
                    else:
                        So = sop.tile([H, BS], f32, tag="So", name="So")
                    nc.scalar.activation(So, zo, _ACT.Sigmoid)
                    # weave next-chunk xz pieces
                    if t + CHUNK < T:
                        if k == 0:
                            pz_next = alloc_pz_ilv()
                        if k < NP:
                            emit_piece(pz_next, (t // CHUNK + 1) * CHUNK, k)
                    # cell update
                    if static:
                        ig2 = ig2_ab[t % 2]
                        fcv = fc_ab[t % 2]
                        th = th_ab[t % 2]
                    else:
                        ig2 = tmps.tile([H, BS], f32, tag="ig2", name="ig2")
                        fcv = tmps.tile([H, BS], f32, tag="fc", name="fc")
                        th = tmps.tile([H, BS], f32, tag="th", name="th")
                    cell_op.scalar_tensor_tensor(
                        ig2, sg, 0.5, si, _ALU.subtract, _ALU.mult)
                    fc_op.tensor_mul(fcv, sf, c)
                    cell_op.scalar_tensor_tensor(
                        c, ig2, 2.0, fcv, _ALU.mult, _ALU.add)
                    nc.scalar.activation(th, c, _ACT.Tanh)
                    last_step = last_layer and t == T - 1
                    if last_step:
                        ho_op.tensor_mul(hc_stage[:, 0:BS], th, So)
                    else:
                        if layer == 0:
                            h_prev = hs0[:, t * BS:(t + 1) * BS]
                        elif static:
                            h_prev = h1_ab[t % 2]
                        else:
                            h_prev = hsmall.tile([H, BS], bf, tag="h1",
                                                 name="h1")
                        ho_op.tensor_mul(h_prev, th, So)
                    if touch and static and not last_step:
                        t_op = nc.gpsimd if touch == "pool" else nc.vector
                        t_op.memset(S, 0.0)
                        t_op.memset(th, 0.0)
                        t_op.memset(So, 0.0)
                return h_prev

            def emit_layer(layer, x_s, w_s, u_s, b_s, h_prev, last_layer):
                pz_cur = alloc_pz()
                for p in range(NPIECE):
                    emit_gemm_piece(pz_cur, w_s, x_s, 0, p)
                pz_next = None
                for t in range(T):
                    k = t % CHUNK
                    if k == 0 and t > 0:
                        pz_cur = pz_next
                    pz3 = pz_cur[0].rearrange("p (g n) -> p g n", g=NG)
                    sl = slice(k * BS, (k + 1) * BS)
                    # on-chain matmuls: f, g, i (and o if not split)
                    for j in range(NG):
                        last = j == NG - 1
                        nc.tensor.matmul(
                            pz3[:, j, sl],
                            u_s[:, j * H:(j + 1) * H],
                            h_prev,
                            start=False,
                            stop=last and b_s is None,
                            skip_group_check=True,
                        )
                        if b_s is not None:
                            nc.tensor.matmul(
                                pz3[:, j, sl],
                                b_s[:, j * H:(j + 1) * H],
                                ones,
                                start=False, stop=last,
                                skip_group_check=True,
                            )
                    # sigma over (f, g, i)  [128, NG*BS]
                    if static:
                        S = S_ab[t % 2]
                    else:
                        S = gates.tile([H, NG * BS], f32, tag="S")
                    S3 = S.rearrange("p (g n) -> p g n", g=NG)
                    nc.scalar.activation(S3, pz3[:, 0:NG, sl], _ACT.Sigmoid)
                    sf = S[:, 0:BS]
                    sg = S[:, BS:2 * BS]
                    si = S[:, 2 * BS:3 * BS]
                    if split_o:
                        # o-gate matmul + sigma(o), off the chain (own bank)
                        zo = pz_cur[1][:, sl]
                        nc.tensor.matmul(
                            zo, u_s[:, 3 * H:4 * H], h_prev,
                            start=False, stop=b_s is None,
                            skip_group_check=True,
                        )
                        if b_s is not None:
                            nc.tensor.matmul(
                                zo, b_s[:, 3 * H:4 * H], ones,
                                start=False, stop=True,
                                skip_group_check=True,
                            )
                        if static:
                            So = So_ab[t % 2]
                        else:
                            So = sop.tile([H, BS], f32, tag="So")
                        nc.scalar.activation(So, zo, _ACT.Sigmoid)
                    else:
                        So = S[:, 3 * BS:4 * BS]
                    # xz piece for the chunk after next (uniform PE cadence)
                    if t + CHUNK < T:
                        if k == 0:
                            pz_next = alloc_pz()
                        emit_gemm_piece(pz_next, w_s, x_s,
                                        (t // CHUNK + 1) * CHUNK, k)
                    # cell update
                    if static:
                        ig2 = ig2_ab[t % 2]
                        fcv = fc_ab[t % 2]
                        th = th_ab[t % 2]
                    else:
                        ig2 = tmps.tile([H, BS], f32, tag="ig2")
                        fcv = tmps.tile([H, BS], f32, tag="fc")
                        th = tmps.tile([H, BS], f32, tag="th")
                    cell_op.scalar_tensor_tensor(
                        ig2, sg, 0.5, si, _ALU.subtract, _ALU.mult)
                    fc_op.tensor_mul(fcv, sf, c)
                    cell_op.scalar_tensor_tensor(
                        c, ig2, 2.0, fcv, _ALU.mult, _ALU.add)
                    nc.scalar.activation(th, c, _ACT.Tanh)
                    last_step = last_layer and t == T - 1
                    if last_step:
                        cell_op.tensor_mul(hc_stage[:, 0:BS], th, So)
                    else:
                        if layer == 0:
                            h_prev = hs0[:, t * BS:(t + 1) * BS]
                        elif static:
                            h_prev = h1_ab[t % 2]
                        else:
                            h_prev = hsmall.tile([H, BS], bf, tag="h1")
                        cell_op.tensor_mul(h_prev, th, So)
                    if touch and static and not last_step:
                        # overwrite dead ACT-written tiles so the next ACT
                        # write's WAW dep is a cross-engine (covered) tick
                        t_op = nc.gpsimd if touch == "pool" else nc.vector
                        t_op.memset(S, 0.0)
                        t_op.memset(th, 0.0)
                        if split_o:
                            t_op.memset(So, 0.0)
                return h_prev

            def body():
                nc.vector.memset(c, 0.0)
                nc.vector.memset(h0, 0.0)
                if perstep:
                    emitter = emit_layer_perstep
                elif ilv:
                    emitter = emit_layer_ilv
                else:
                    emitter = emit_layer
                hlast0 = emitter(0, xTs, w0s, u0s, None, h0,
                                 last_layer=False)
                emitter(1, hs0, w1s, u1s, b1s, hlast0, last_layer=True)
                nc.vector.tensor_copy(hc_stage[:, BS:2 * BS], c)
                nc.sync.dma_start(out=hc, in_=hc_stage)

            if reps == 1:
                body()
            else:
                with tc.For_i(0, reps, 1):
                    body()

    nc.finalize()
    return nc


_CACHE = {}


def _get_program(T, has_b1, reps=1, cfg=None):
    key = (T, has_b1, reps, tuple(sorted((cfg or {}).items())))
    if key not in _CACHE:
        _CACHE[key] = _build(T, has_b1, reps, cfg)
    return _CACHE[key]


def _prep_weights(W0, U0, b0, W1, U1, b1):
    """Permute gates to (f, g, i, o), scale g-block by 2, cast bf16."""
    def prep(M):
        Mp = np.asarray(M, np.float32)[..., _PERM].copy()
        Mp[..., H:2 * H] *= 2.0
        return Mp
    w0a = np.concatenate([prep(W0), prep(b0)[None, :]], axis=0).astype(BF16)
    u0a = prep(U0).astype(BF16)
    w1a = prep(W1).astype(BF16)
    u1a = prep(U1).astype(BF16)
    b1p = prep(b1)[None, :].astype(BF16)
    has_b1 = bool(np.any(np.asarray(b1) != 0))
    return w0a, u0a, w1a, u1a, b1p, has_b1


def _prep_x(enc_inp, T):
    outs = []
    for k in range(N_CORES):
        xk = np.asarray(enc_inp[k * BS:(k + 1) * BS, :T], np.float32)
        xk = np.ascontiguousarray(xk.transpose(2, 1, 0)).reshape(F, T * BS)
        xa = np.concatenate([xk, np.ones((1, T * BS), np.float32)], axis=0)
        outs.append(xa.astype(BF16))
    return outs


def run_lstm(enc_inp, W0, U0, b0, W1, U1, b1, T=T_FULL, reps=1, pads=None,
             cfg=None):
    w0a, u0a, w1a, u1a, b1p, has_b1 = _prep_weights(W0, U0, b0, W1, U1, b1)
    xs = _prep_x(enc_inp, T)
    nc = _get_program(T, has_b1, reps, cfg)
    in_maps = []
    for k in range(N_CORES):
        m = {"xT": xs[k], "w0": w0a, "u0": u0a, "w1": w1a, "u1": u1a}
        if has_b1:
            m["b1"] = b1p
        in_maps.append(m)
    res = run_bass_kernel_spmd(nc, in_maps, list(range(N_CORES)))
    h = np.empty((B, H), np.float32)
    c = np.empty((B, H), np.float32)
    for k in range(N_CORES):
        hck = res.results[k]["hc"]
        h[k * BS:(k + 1) * BS] = hck[:, :BS].T
        c[k * BS:(k + 1) * BS] = hck[:, BS:].T
    return h, c


def kernel(enc_inp, W0, U0, b0, W1, U1, b1):
    h, c = run_lstm(np.asarray(enc_inp), np.asarray(W0), np.asarray(U0),
                    np.asarray(b0), np.asarray(W1), np.asarray(U1),
                    np.asarray(b1), T=T_FULL)
    return h, c


# revision 7
# speedup vs baseline: 1.0714x; 1.0714x over previous
"""Trainium2 Bass kernel for a 2-layer LSTM encoder returning final (h, c).

Problem: enc_inp [B=128, T=1024, F=64]; two stacked LSTM layers with H=128;
layer 2's initial state is layer 1's final state, so the 2048 recurrence
steps are strictly sequential and the kernel is bound by the per-step
cross-engine dependency chain  PE(U@h) -> ACT(sigmoid) -> DVE(cell) ->
ACT(tanh) -> DVE(h) -> PE.

Chain-latency optimizations vs the padded predecessor (HW-validated ~11%
faster in an interleaved A/B; sim + HW agree on the structure):
 - No filler/pad ops (Tile's scheduler displaced them from their intended
   per-step slots anyway, so they only added engine busy-work).
 - Gate column order (f, g2, i, o): only the f,g,i matmuls + one 48-wide
   sigmoid sit on the critical path. The o-gate matmul accumulates into its
   OWN psum bank (separate tile - avoids Tile's same-bank write-after-read
   serialization against the 3-gate sigmoid) and sigma(o) runs in the
   shadow of the DVE cell update.
 - tanh(g) via 2*sigmoid(2x)-1 with g-gate weights pre-scaled by 2 (host),
   so the gate activation is a single Sigmoid op; cell update is 3 fused
   DVE ops (stt/tt/stt); tanh(c) is the only other ACT op; h in bf16.
 - Static parity-pair (t%2) tiles for all per-step tensors instead of
   rotating pools: WAR/WAW deps resolve to ticks already covered by each
   engine's observed vector clock, so chain instructions carry their real
   cross-engine wait attached (fewer sequencer-blocking EventSemaphores).
Rejected on HW measurement: per-step psum tiles (bank cycling + extra
LDWEIGHTS cost ~+900ns/step), GPSIMD offload of fc/cell ops (stt fails to
compile; fc-on-pool measured +48ns/step in an interleaved A/B - GPSIMD
dispatch + the DVE-shared SBUF port cost more than the overlap saves),
SEQ-pacing pads, bank-interleaved step layout (sim-neutral).
"""

import numpy as np
import ml_dtypes

import concourse.bacc as bacc
import concourse.tile as tile
import concourse.mybir as mybir
from concourse.bass_utils import run_bass_kernel_spmd

N_CORES = 8
B, T_FULL, F, H = 128, 1024, 64, 128
BS = B // N_CORES
G4 = 4 * H

BF16 = ml_dtypes.bfloat16

# keras gate order (i, f, g, o) -> ours (f, g, i, o); g block scaled by 2
_PERM = np.concatenate(
    [np.arange(H, 2 * H), np.arange(2 * H, 3 * H), np.arange(0, H),
     np.arange(3 * H, 4 * H)]
)

_ALU = mybir.AluOpType
_ACT = mybir.ActivationFunctionType

DEFAULT_CFG = {
    "fc": "dve",       # "dve" | "pool"  (engine for fc = sigma(f)*c)
    "ho": "dve",       # "dve" | "pool"  (engine for h = tanh(c)*sigma(o))
    "cell": "dve",     # "dve" | "pool"  (engine for ig2/c/ho)
    "split_o": True,    # o-gate matmul + sigmoid off the critical path
    "static": True,     # parity-pair static tiles instead of pool rotation
    "touch": None,      # None | "dve": overwrite dead ACT-written tiles so
                        # the next ACT write's WAW dep is a covered cross tick
                        # (HW A/B inconclusive; default off)
    "bufs": 8,
    "npar": 4,         # static tile cycle length; 4 keeps tanh's cross wait
                       # attached (no sequencer-blocking EventSemaphore):
                       # HW A/B 1800 vs 1883 ns/step against npar=2
    "chunk": 16,
    "qw": 64,
    "perstep": False,   # per-step psum z tiles (bank rotation) + lookahead xz
    "look": 4,          # xz lookahead steps in perstep mode
    "pz_bufs": 6,       # per-step z tile pool depth (each = 1 psum bank)
    "ilv": False,       # interleave steps across the 2 banks of the chunked
                        # pz tile (even steps bank0, odd bank1) so consecutive
                        # sigmas read different banks and the PE wait attaches
}


def _build(T, has_b1, reps=1, cfg=None):
    cfg = dict(DEFAULT_CFG, **(cfg or {}))
    CHUNK = cfg["chunk"]
    QW = cfg["qw"]
    bufs = cfg["bufs"]
    split_o = cfg["split_o"]
    fc_eng = cfg["fc"]
    cell_eng = cfg["cell"]
    static = cfg["static"]
    NPAR = cfg["npar"]
    touch = cfg["touch"]
    perstep = cfg["perstep"]
    LOOK = cfg["look"]
    pz_bufs = cfg["pz_bufs"]
    ilv = cfg["ilv"]
    if perstep:
        assert split_o and pz_bufs >= LOOK + 2
    if ilv:
        assert split_o and CHUNK == 16 and BS == 16

    bf = mybir.dt.bfloat16
    f32 = mybir.dt.float32

    nc = bacc.Bacc("TRN2", target_bir_lowering=False, debug=False,
                   enable_asserts=True, num_devices=N_CORES)

    xT = nc.dram_tensor("xT", [F + 1, T * BS], bf, kind="ExternalInput").ap()
    w0 = nc.dram_tensor("w0", [F + 1, G4], bf, kind="ExternalInput").ap()
    u0 = nc.dram_tensor("u0", [H, G4], bf, kind="ExternalInput").ap()
    w1 = nc.dram_tensor("w1", [H, G4], bf, kind="ExternalInput").ap()
    u1 = nc.dram_tensor("u1", [H, G4], bf, kind="ExternalInput").ap()
    if has_b1:
        b1 = nc.dram_tensor("b1", [1, G4], bf, kind="ExternalInput").ap()
    hc = nc.dram_tensor("hc", [H, 2 * BS], f32, kind="ExternalOutput").ap()

    assert T % CHUNK == 0
    PZW = 4 * CHUNK * BS
    NPIECE = CHUNK
    PPG = CHUNK * BS // QW
    assert PPG * 4 == NPIECE

    with tile.TileContext(nc) as tc:
        with (
            tc.tile_pool(name="big", bufs=1) as big,
            tc.tile_pool(name="wts", bufs=1) as wts,
            tc.tile_pool(name="state", bufs=1) as state,
            tc.tile_pool(name="gates", bufs=bufs) as gates,
            tc.tile_pool(name="sop", bufs=bufs) as sop,
            tc.tile_pool(name="tmps", bufs=bufs) as tmps,
            tc.tile_pool(name="hsmall", bufs=bufs) as hsmall,
            tc.tile_pool(name="pz", bufs=2, space="PSUM") as pzpool,
            tc.tile_pool(name="pzo", bufs=2, space="PSUM") as pzopool,
            tc.tile_pool(name="pzs", bufs=pz_bufs, space="PSUM") as pzs,
        ):
            xTs = big.tile([F + 1, T * BS], bf, tag="xT")
            nc.sync.dma_start(out=xTs, in_=xT)
            hs0 = big.tile([H, T * BS], bf, tag="hs0")

            w0s = wts.tile([F + 1, G4], bf, tag="w0")
            u0s = wts.tile([H, G4], bf, tag="u0")
            w1s = wts.tile([H, G4], bf, tag="w1")
            u1s = wts.tile([H, G4], bf, tag="u1")
            nc.sync.dma_start(out=w0s, in_=w0)
            nc.sync.dma_start(out=u0s, in_=u0)
            nc.sync.dma_start(out=w1s, in_=w1)
            nc.sync.dma_start(out=u1s, in_=u1)
            b1s = None
            ones = None
            if has_b1:
                b1s = wts.tile([1, G4], bf, tag="b1")
                nc.sync.dma_start(out=b1s, in_=b1)
                ones = state.tile([1, BS], bf, tag="ones")
                nc.vector.memset(ones, 1.0)

            c = state.tile([H, BS], f32, tag="c")
            h0 = state.tile([H, BS], bf, tag="h0")
            hc_stage = state.tile([H, 2 * BS], f32, tag="hc_stage")

            fc_op = nc.gpsimd if fc_eng == "pool" else nc.vector
            cell_op = nc.gpsimd if cell_eng == "pool" else nc.vector
            ho_op = nc.gpsimd if cfg["ho"] == "pool" else nc.vector

            NG = 3 if split_o else 4   # gates in the main psum tile

            if static:
                # parity-cycle static tiles: step t uses index t % NPAR
                S_ab = [state.tile([H, NG * BS], f32, tag=f"S{p}", name=f"S{p}")
                        for p in range(NPAR)]
                So_ab = ([state.tile([H, BS], f32, tag=f"So{p}", name=f"So{p}")
                          for p in range(NPAR)] if split_o else None)
                ig2_ab = [state.tile([H, BS], f32, tag=f"ig2{p}", name=f"ig2{p}")
                          for p in range(NPAR)]
                fc_ab = [state.tile([H, BS], f32, tag=f"fc{p}", name=f"fc{p}")
                         for p in range(NPAR)]
                th_ab = [state.tile([H, BS], f32, tag=f"th{p}", name=f"th{p}")
                         for p in range(NPAR)]
                h1_ab = [state.tile([H, BS], bf, tag=f"h1{p}", name=f"h1{p}")
                         for p in range(NPAR)]
            PZW_M = NG * CHUNK * BS

            def emit_gemm_piece(pzs, w_s, x_s, c0, piece):
                """pzs = (pz_main, pz_o or None)"""
                j, q = piece // PPG, piece % PPG
                cols = slice(c0 * BS + q * QW, c0 * BS + (q + 1) * QW)
                if split_o and j == 3:
                    dst = pzs[1][:, q * QW:(q + 1) * QW]
                    start = q == 0
                else:
                    pz3 = pzs[0].rearrange("p (g n) -> p g n", g=NG)
                    dst = pz3[:, j, q * QW:(q + 1) * QW]
                    start = q == 0 and j % 2 == 0
                nc.tensor.matmul(
                    dst, w_s[:, j * H:(j + 1) * H], x_s[:, cols],
                    start=start, stop=False, skip_group_check=True,
                )

            def alloc_pz():
                pz = pzpool.tile([H, PZW_M], f32, tag="pz")
                if split_o:
                    pzo = pzopool.tile([H, CHUNK * BS], f32, tag="pzo")
                else:
                    pzo = None
                return (pz, pzo)

            def emit_layer_perstep(layer, x_s, w_s, u_s, b_s, h_prev,
                                   last_layer):
                """Per-step z tiles in rotating psum banks; o-gate chunked."""
                zt = {}

                def emit_xz_step(tp):
                    z = pzs.tile([H, 3 * BS], f32, tag="z", name="z")
                    zt[tp] = z
                    for j in range(3):
                        nc.tensor.matmul(
                            z[:, j * BS:(j + 1) * BS],
                            w_s[:, j * H:(j + 1) * H],
                            x_s[:, tp * BS:(tp + 1) * BS],
                            start=(j == 0), stop=False, skip_group_check=True,
                        )

                def emit_xzo_chunk(pzo, c0):
                    for q in range(PPG):
                        cols = slice(c0 * BS + q * QW, c0 * BS + (q + 1) * QW)
                        nc.tensor.matmul(
                            pzo[:, q * QW:(q + 1) * QW],
                            w_s[:, 3 * H:4 * H], x_s[:, cols],
                            start=(q == 0), stop=False, skip_group_check=True,
                        )

                for tp in range(min(LOOK, T)):
                    emit_xz_step(tp)
                pzo_cur = None
                for t in range(T):
                    k = t % CHUNK
                    if k == 0:
                        pzo_cur = pzopool.tile([H, CHUNK * BS], f32,
                                               tag="pzo", name="pzo")
                        emit_xzo_chunk(pzo_cur, t)
                    sl = slice(k * BS, (k + 1) * BS)
                    z_cur = zt.pop(t)
                    for j in range(3):
                        nc.tensor.matmul(
                            z_cur[:, j * BS:(j + 1) * BS],
                            u_s[:, j * H:(j + 1) * H],
                            h_prev,
                            start=False, stop=(j == 2 and b_s is None),
                            skip_group_check=True,
                        )
                        if b_s is not None:
                            nc.tensor.matmul(
                                z_cur[:, j * BS:(j + 1) * BS],
                                b_s[:, j * H:(j + 1) * H], ones,
                                start=False, stop=(j == 2),
                                skip_group_check=True,
                            )
                    if static:
                        S = S_ab[t % NPAR]
                    else:
                        S = gates.tile([H, 3 * BS], f32, tag="S", name="S")
                    nc.scalar.activation(S, z_cur, _ACT.Sigmoid)
                    sf = S[:, 0:BS]
                    sg = S[:, BS:2 * BS]
                    si = S[:, 2 * BS:3 * BS]
                    # o-gate matmul + sigma(o), off the chain
                    zo = pzo_cur[:, sl]
                    nc.tensor.matmul(
                        zo, u_s[:, 3 * H:4 * H], h_prev,
                        start=False, stop=b_s is None, skip_group_check=True,
                    )
                    if b_s is not None:
                        nc.tensor.matmul(
                            zo, b_s[:, 3 * H:4 * H], ones,
                            start=False, stop=True, skip_group_check=True,
                        )
                    if static:
                        So = So_ab[t % NPAR]
                    else:
                        So = sop.tile([H, BS], f32, tag="So", name="So")
                    nc.scalar.activation(So, zo, _ACT.Sigmoid)
                    # xz for step t+LOOK
                    if t + LOOK < T:
                        emit_xz_step(t + LOOK)
                    # cell update
                    if static:
                        ig2 = ig2_ab[t % NPAR]
                        fcv = fc_ab[t % NPAR]
                        th = th_ab[t % NPAR]
                    else:
                        ig2 = tmps.tile([H, BS], f32, tag="ig2", name="ig2")
                        fcv = tmps.tile([H, BS], f32, tag="fc", name="fc")
                        th = tmps.tile([H, BS], f32, tag="th", name="th")
                    cell_op.scalar_tensor_tensor(
                        ig2, sg, 0.5, si, _ALU.subtract, _ALU.mult)
                    fc_op.tensor_mul(fcv, sf, c)
                    cell_op.scalar_tensor_tensor(
                        c, ig2, 2.0, fcv, _ALU.mult, _ALU.add)
                    nc.scalar.activation(th, c, _ACT.Tanh)
                    last_step = last_layer and t == T - 1
                    if last_step:
                        ho_op.tensor_mul(hc_stage[:, 0:BS], th, So)
                    else:
                        if layer == 0:
                            h_prev = hs0[:, t * BS:(t + 1) * BS]
                        elif static:
                            h_prev = h1_ab[t % NPAR]
                        else:
                            h_prev = hsmall.tile([H, BS], bf, tag="h1",
                                                 name="h1")
                        ho_op.tensor_mul(h_prev, th, So)
                    if touch and static and not last_step:
                        t_op = nc.gpsimd if touch == "pool" else nc.vector
                        t_op.memset(S, 0.0)
                        t_op.memset(th, 0.0)
                        t_op.memset(So, 0.0)
                return h_prev

            def emit_layer_ilv(layer, x_s, w_s, u_s, b_s, h_prev, last_layer):
                """Chunked pz with step-parity bank interleave.

                pz tile [H, 1024] = 2 psum banks; step k of the chunk lives at
                [par=k%2, slot=k//2, 0:48] so consecutive sigmas read
                different banks (no same-bank R-R serialization). xz for the
                next chunk is woven one piece per step (6 main + PPG o-gate
                pieces per chunk); each piece covers one (gate, parity).
                """
                NSLOT = CHUNK // 2

                def pz4(pz):
                    return pz.rearrange("p (par slot gb) -> p par slot gb",
                                        par=2, slot=NSLOT)

                def emit_piece(pzs_, c0, piece):
                    if piece < 6:
                        j, par = piece // 2, piece % 2
                        xv = x_s[:, c0 * BS:(c0 + CHUNK) * BS].rearrange(
                            "p (slot par b) -> p par slot b", par=2, b=BS)
                        nc.tensor.matmul(
                            pz4(pzs_[0])[:, par, :, j * BS:(j + 1) * BS],
                            w_s[:, j * H:(j + 1) * H],
                            xv[:, par],
                            start=(j == 0), stop=False, skip_group_check=True,
                        )
                    else:
                        q = piece - 6
                        cols = slice(c0 * BS + q * QW, c0 * BS + (q + 1) * QW)
                        nc.tensor.matmul(
                            pzs_[1][:, q * QW:(q + 1) * QW],
                            w_s[:, 3 * H:4 * H], x_s[:, cols],
                            start=(q == 0), stop=False, skip_group_check=True,
                        )

                NP = 6 + PPG

                def alloc_pz_ilv():
                    pz = pzpool.tile([H, 2 * NSLOT * 64], f32, tag="pz",
                                     name="pz")
                    pzo = pzopool.tile([H, CHUNK * BS], f32, tag="pzo",
                                       name="pzo")
                    return (pz, pzo)

                pz_cur = alloc_pz_ilv()
                for p in range(NP):
                    emit_piece(pz_cur, 0, p)
                pz_next = None
                for t in range(T):
                    k = t % CHUNK
                    if k == 0 and t > 0:
                        pz_cur = pz_next
                    zb = pz4(pz_cur[0])[:, k % 2, k // 2]
                    for j in range(3):
                        nc.tensor.matmul(
                            zb[:, j * BS:(j + 1) * BS],
                            u_s[:, j * H:(j + 1) * H],
                            h_prev,
                            start=False, stop=(j == 2 and b_s is None),
                            skip_group_check=True,
                        )
                        if b_s is not None:
                            nc.tensor.matmul(
                                zb[:, j * BS:(j + 1) * BS],
                                b_s[:, j * H:(j + 1) * H], ones,
                                start=False, stop=(j == 2),
                                skip_group_check=True,
                            )
                    if static:
                        S = S_ab[t % NPAR]
                    else:
                        S = gates.tile([H, 3 * BS], f32, tag="S", name="S")
                    nc.scalar.activation(S, zb[:, 0:3 * BS], _ACT.Sigmoid)
                    sf = S[:, 0:BS]
                    sg = S[:, BS:2 * BS]
                    si = S[:, 2 * BS:3 * BS]
                    zo = pz_cur[1][:, k * BS:(k + 1) * BS]
                    nc.tensor.matmul(
                        zo, u_s[:, 3 * H:4 * H], h_prev,
                        start=False, stop=b_s is None, skip_group_check=True,
                    )
                    if b_s is not None:
                        nc.tensor.matmul(
                            zo, b_s[:, 3 * H:4 * H], ones,
                            start=False, stop=True, skip_group_check=True,
                        )
                    if static:
                        So = So_ab[t % NPAR]
                    else:
                        So = sop.tile([H, BS], f32, tag="So", name="So")
                    nc.scalar.activation(So, zo, _ACT.Sigmoid)
                    # weave next-chunk xz pieces
                    if t + CHUNK < T:
                        if k == 0:
                            pz_next = alloc_pz_ilv()
                        if k < NP:
                            emit_piece(pz_next, (t // CHUNK + 1) * CHUNK, k)
                    # cell update
                    if static:
                        ig2 = ig2_ab[t % NPAR]
                        fcv = fc_ab[t % NPAR]
                        th = th_ab[t % NPAR]
                    else:
                        ig2 = tmps.tile([H, BS], f32, tag="ig2", name="ig2")
                        fcv = tmps.tile([H, BS], f32, tag="fc", name="fc")
                        th = tmps.tile([H, BS], f32, tag="th", name="th")
                    cell_op.scalar_tensor_tensor(
                        ig2, sg, 0.5, si, _ALU.subtract, _ALU.mult)
                    fc_op.tensor_mul(fcv, sf, c)
                    cell_op.scalar_tensor_tensor(
                        c, ig2, 2.0, fcv, _ALU.mult, _ALU.add)
                    nc.scalar.activation(th, c, _ACT.Tanh)
                    last_step = last_layer and t == T - 1
                    if last_step:
                        ho_op.tensor_mul(hc_stage[:, 0:BS], th, So)
                    else:
                        if layer == 0:
                            h_prev = hs0[:, t * BS:(t + 1) * BS]
                        elif static:
                            h_prev = h1_ab[t % NPAR]
                        else:
                            h_prev = hsmall.tile([H, BS], bf, tag="h1",
                                                 name="h1")
                        ho_op.tensor_mul(h_prev, th, So)
                    if touch and static and not last_step:
                        t_op = nc.gpsimd if touch == "pool" else nc.vector
                        t_op.memset(S, 0.0)
                        t_op.memset(th, 0.0)
                        t_op.memset(So, 0.0)
                return h_prev

            def emit_layer(layer, x_s, w_s, u_s, b_s, h_prev, last_layer):
                pz_cur = alloc_pz()
                for p in range(NPIECE):
                    emit_gemm_piece(pz_cur, w_s, x_s, 0, p)
                pz_next = None
                for t in range(T):
                    k = t % CHUNK
                    if k == 0 and t > 0:
                        pz_cur = pz_next
                    pz3 = pz_cur[0].rearrange("p (g n) -> p g n", g=NG)
                    sl = slice(k * BS, (k + 1) * BS)
                    # on-chain matmuls: f, g, i (and o if not split)
                    for j in range(NG):
                        last = j == NG - 1
                        nc.tensor.matmul(
                            pz3[:, j, sl],
                            u_s[:, j * H:(j + 1) * H],
                            h_prev,
                            start=False,
                            stop=last and b_s is None,
                            skip_group_check=True,
                        )
                        if b_s is not None:
                            nc.tensor.matmul(
                                pz3[:, j, sl],
                                b_s[:, j * H:(j + 1) * H],
                                ones,
                                start=False, stop=last,
                                skip_group_check=True,
                            )
                    # sigma over (f, g, i)  [128, NG*BS]
                    if static:
                        S = S_ab[t % NPAR]
                    else:
                        S = gates.tile([H, NG * BS], f32, tag="S")
                    S3 = S.rearrange("p (g n) -> p g n", g=NG)
                    nc.scalar.activation(S3, pz3[:, 0:NG, sl], _ACT.Sigmoid)
                    sf = S[:, 0:BS]
                    sg = S[:, BS:2 * BS]
                    si = S[:, 2 * BS:3 * BS]
                    if split_o:
                        # o-gate matmul + sigma(o), off the chain (own bank)
                        zo = pz_cur[1][:, sl]
                        nc.tensor.matmul(
                            zo, u_s[:, 3 * H:4 * H], h_prev,
                            start=False, stop=b_s is None,
                            skip_group_check=True,
                        )
                        if b_s is not None:
                            nc.tensor.matmul(
                                zo, b_s[:, 3 * H:4 * H], ones,
                                start=False, stop=True,
                                skip_group_check=True,
                            )
                        if static:
                            So = So_ab[t % NPAR]
                        else:
                            So = sop.tile([H, BS], f32, tag="So")
                        nc.scalar.activation(So, zo, _ACT.Sigmoid)
                    else:
                        So = S[:, 3 * BS:4 * BS]
                    # xz piece for the chunk after next (uniform PE cadence)
                    if t + CHUNK < T:
                        if k == 0:
                            pz_next = alloc_pz()
                        emit_gemm_piece(pz_next, w_s, x_s,
                                        (t // CHUNK + 1) * CHUNK, k)
                    # cell update
                    if static:
                        ig2 = ig2_ab[t % NPAR]
                        fcv = fc_ab[t % NPAR]
                        th = th_ab[t % NPAR]
                    else:
                        ig2 = tmps.tile([H, BS], f32, tag="ig2")
                        fcv = tmps.tile([H, BS], f32, tag="fc")
                        th = tmps.tile([H, BS], f32, tag="th")
                    cell_op.scalar_tensor_tensor(
                        ig2, sg, 0.5, si, _ALU.subtract, _ALU.mult)
                    fc_op.tensor_mul(fcv, sf, c)
                    cell_op.scalar_tensor_tensor(
                        c, ig2, 2.0, fcv, _ALU.mult, _ALU.add)
                    nc.scalar.activation(th, c, _ACT.Tanh)
                    last_step = last_layer and t == T - 1
                    if last_step:
                        cell_op.tensor_mul(hc_stage[:, 0:BS], th, So)
                    else:
                        if layer == 0:
                            h_prev = hs0[:, t * BS:(t + 1) * BS]
                        elif static:
                            h_prev = h1_ab[t % NPAR]
                        else:
                            h_prev = hsmall.tile([H, BS], bf, tag="h1")
                        cell_op.tensor_mul(h_prev, th, So)
                    if touch and static and not last_step:
                        # overwrite dead ACT-written tiles so the next ACT
                        # write's WAW dep is a cross-engine (covered) tick
                        t_op = nc.gpsimd if touch == "pool" else nc.vector
                        t_op.memset(S, 0.0)
                        t_op.memset(th, 0.0)
                        if split_o:
                            t_op.memset(So, 0.0)
                return h_prev

            def body():
                nc.vector.memset(c, 0.0)
                nc.vector.memset(h0, 0.0)
                if perstep:
                    emitter = emit_layer_perstep
                elif ilv:
                    emitter = emit_layer_ilv
                else:
                    emitter = emit_layer
                hlast0 = emitter(0, xTs, w0s, u0s, None, h0,
                                 last_layer=False)
                emitter(1, hs0, w1s, u1s, b1s, hlast0, last_layer=True)
                nc.vector.tensor_copy(hc_stage[:, BS:2 * BS], c)
                nc.sync.dma_start(out=hc, in_=hc_stage)

            if reps == 1:
                body()
            else:
                with tc.For_i(0, reps, 1):
                    body()

    nc.finalize()
    return nc


_CACHE = {}


def _get_program(T, has_b1, reps=1, cfg=None):
    key = (T, has_b1, reps, tuple(sorted((cfg or {}).items())))
    if key not in _CACHE:
        _CACHE[key] = _build(T, has_b1, reps, cfg)
    return _CACHE[key]


def _prep_weights(W0, U0, b0, W1, U1, b1):
    """Permute gates to (f, g, i, o), scale g-block by 2, cast bf16."""
    def prep(M):
        Mp = np.asarray(M, np.float32)[..., _PERM].copy()
        Mp[..., H:2 * H] *= 2.0
        return Mp
    w0a = np.concatenate([prep(W0), prep(b0)[None, :]], axis=0).astype(BF16)
    u0a = prep(U0).astype(BF16)
    w1a = prep(W1).astype(BF16)
    u1a = prep(U1).astype(BF16)
    b1p = prep(b1)[None, :].astype(BF16)
    has_b1 = bool(np.any(np.asarray(b1) != 0))
    return w0a, u0a, w1a, u1a, b1p, has_b1


def _prep_x(enc_inp, T):
    outs = []
    for k in range(N_CORES):
        xk = np.asarray(enc_inp[k * BS:(k + 1) * BS, :T], np.float32)
        xk = np.ascontiguousarray(xk.transpose(2, 1, 0)).reshape(F, T * BS)
        xa = np.concatenate([xk, np.ones((1, T * BS), np.float32)], axis=0)
        outs.append(xa.astype(BF16))
    return outs


def run_lstm(enc_inp, W0, U0, b0, W1, U1, b1, T=T_FULL, reps=1, pads=None,
             cfg=None):
    w0a, u0a, w1a, u1a, b1p, has_b1 = _prep_weights(W0, U0, b0, W1, U1, b1)
    xs = _prep_x(enc_inp, T)
    nc = _get_program(T, has_b1, reps, cfg)
    in_maps = []
    for k in range(N_CORES):
        m = {"xT": xs[k], "w0": w0a, "u0": u0a, "w1": w1a, "u1": u1a}
        if has_b1:
            m["b1"] = b1p
        in_maps.append(m)
    res = run_bass_kernel_spmd(nc, in_maps, list(range(N_CORES)))
    h = np.empty((B, H), np.float32)
    c = np.empty((B, H), np.float32)
    for k in range(N_CORES):
        hck = res.results[k]["hc"]
        h[k * BS:(k + 1) * BS] = hck[:, :BS].T
        c[k * BS:(k + 1) * BS] = hck[:, BS:].T
    return h, c


def kernel(enc_inp, W0, U0, b0, W1, U1, b1):
    h, c = run_lstm(np.asarray(enc_inp), np.asarray(W0), np.asarray(U0),
                    np.asarray(b0), np.asarray(W1), np.asarray(U1),
                    np.asarray(b1), T=T_FULL)
    return h, c
